# revision 2
# baseline (speedup 1.0000x reference)
"""Trainium2 Bass kernel for ModLinear forward (fp16 GEMM, transposed I/O).

Math: alpha folds into a per-batch modulated weight (wmodT[i, o] =
weight[o, i] * alpha[b, i]); the device runs outT = wmodT.T @ xT as a
pure fp16 GEMM (f32 PSUM accumulation), beta added during the
PSUM->SBUF downcast; host pre-transposes x to fp16 xT per core and
un-transposes/upcasts the fp16 outT. Max-rel-err vs the f32 reference
~4.3e-4 (fp16 rounding on x/w/out; gate is 2e-2).

Pipeline (learned from v2/v3 hardware traces):
  - The PE matmul cadence floor is 213 ns per 512-col fp16 matmul
    (1 col/cycle @ 2.4 GHz); v2 measured 216 ns with PSUM drained
    exclusively by the scalar/ACT engine.
  - v3 showed draining half the PSUM banks with DVE tensor_add slows
    EVERY matmul by 43 ns (PSUM port contention) -> drain only on ACT.
  - Tapered window schedule [1024, 2048, 7x4096, 1024]: PE starts
    after a 1 MiB load instead of 4 MiB, and the final drain is 1 MiB
    of out-DMA instead of 4 MiB.
  - Weights pre-packed host-side into the exact SBUF layout
    [128, 4*512] -> one DMA with 4 KiB descriptors (instead of 512
    1-KiB packets serialized ahead of the first x slab).

Per core: 1024 matmuls x 512 cols = 223 us PE-bound + ~22 us fixed
(framework start/teardown barriers + fill/drain); DMA 64 MiB total is
~165 us engine-busy, fully hidden. Measured best 245.5 us (vs 407 us
for the f32 baseline on the same machine, a 1.66x speedup); rel err
4.3e-4.
"""

import numpy as np

B, N = 2, 131072
IN_F, OUT_F, STYLE_F = 512, 512, 256
NCORES = 8
ROWS = B * N
ROWS_PER_CORE = ROWS // NCORES  # 32768
P = 128
KC = IN_F // P   # 4 contraction chunks
OC = OUT_F // P  # 4 output chunks
SUB = 512        # rows per PSUM sub-block (one PSUM bank)
# Tapered window schedule; sums to ROWS_PER_CORE.
WINDOWS = [1024, 2048] + [4096] * 7 + [1024]
assert sum(WINDOWS) == ROWS_PER_CORE
WMAX = max(WINDOWS)


def _build_body(tc, out_ap, xt_ap, wt_ap, betat_ap):
    from concourse import mybir

    nc = tc.nc
    f32 = mybir.dt.float32
    f16 = mybir.dt.float16

    with (
        tc.tile_pool(name="const", bufs=1) as cpool,
        tc.tile_pool(name="xin", bufs=2) as xpool,
        tc.tile_pool(name="oout", bufs=2) as opool,
        tc.tile_pool(name="ps", bufs=8, space="PSUM") as pspool,
    ):
        # Weights already in SBUF layout -> one fat DMA; betaT is tiny.
        wt_sb = cpool.tile([P, KC * OUT_F], f16)
        nc.sync.dma_start(out=wt_sb[:], in_=wt_ap[:, :])
        beta_sb = cpool.tile([P, OC], f32)
        nc.sync.dma_start(out=beta_sb[:], in_=betat_ap[:, :])

        r0 = 0
        for w, W in enumerate(WINDOWS):
            xs = []
            for k in range(KC):
                xk = xpool.tile([P, WMAX], f16, name=f"xk{k}")
                nc.sync.dma_start(
                    out=xk[:, :W],
                    in_=xt_ap[k * P : (k + 1) * P, r0 : r0 + W],
                )
                xs.append(xk)
            ots = [
                opool.tile([P, WMAX], f16, name=f"ot{c}") for c in range(OC)
            ]

            for sb in range(W // SUB):
                rlo, rhi = sb * SUB, (sb + 1) * SUB
                for c in range(OC):
                    ps = pspool.tile([P, SUB], f32)
                    for k in range(KC):
                        nc.tensor.matmul(
                            ps[:],
                            wt_sb[:, k * OUT_F + c * P : k * OUT_F + (c + 1) * P],
                            xs[k][:, rlo:rhi],
                            start=(k == 0),
                            stop=(k == KC - 1),
                        )
                    # PSUM -> SBUF on ACT only: copy + per-partition beta
                    # bias + fp16 downcast in one op.
                    nc.scalar.add(
                        out=ots[c][:, rlo:rhi],
                        in_=ps[:],
                        add=beta_sb[:, c : c + 1],
                    )

            for c in range(OC):
                nc.scalar.dma_start(
                    out=out_ap[c * P : (c + 1) * P, r0 : r0 + W],
                    in_=ots[c][:, :W],
                )
            r0 += W


def build_nc(rows_per_core=ROWS_PER_CORE):
    """Build + compile the per-core Bass program."""
    import concourse.tile as tile
    from concourse import bacc, mybir

    f16 = mybir.dt.float16
    nc = bacc.Bacc(
        "TRN2", target_bir_lowering=False, debug=False, num_devices=NCORES
    )
    xt_t = nc.dram_tensor("xt", [IN_F, rows_per_core], f16, kind="ExternalInput")
    wt_t = nc.dram_tensor("wt", [P, KC * OUT_F], f16, kind="ExternalInput")
    betat_t = nc.dram_tensor("betat", [P, OC], mybir.dt.float32, kind="ExternalInput")
    out_t = nc.dram_tensor("out", [OUT_F, rows_per_core], f16, kind="ExternalOutput")

    with tile.TileContext(nc) as tc:
        _build_body(tc, out_t.ap(), xt_t.ap(), wt_t.ap(), betat_t.ap())
    nc.compile()
    return nc


_NC_CACHE = {}


def _get_nc(rows_per_core=ROWS_PER_CORE):
    if rows_per_core not in _NC_CACHE:
        _NC_CACHE[rows_per_core] = build_nc(rows_per_core)
    return _NC_CACHE[rows_per_core]


def host_prep(x, z, weight, weight_alpha, bias_alpha, weight_beta, bias_beta):
    """Per-batch packed weights/biases + per-core transposed fp16 x."""
    z64 = z.astype(np.float64)
    alpha = (z64 @ weight_alpha.astype(np.float64).T) + bias_alpha.astype(np.float64)
    beta = (z64 @ weight_beta.astype(np.float64).T) + bias_beta.astype(np.float64)
    alpha = alpha.astype(np.float32)  # [B, IN_F]
    beta = beta.astype(np.float32)  # [B, OUT_F]

    wpack = []
    btpack = []
    for b in range(B):
        # wpack[p, k*OUT_F + o] = weight[o, k*P + p] * alpha[b, k*P + p]
        wmodT = (weight.T * alpha[b][:, None]).astype(np.float16)  # [IN_F, OUT_F]
        wpack.append(
            np.ascontiguousarray(
                wmodT.reshape(KC, P, OUT_F).transpose(1, 0, 2).reshape(P, KC * OUT_F)
            )
        )
        # betaT[p, c] = beta[b, c*P + p]
        btpack.append(
            np.ascontiguousarray(beta[b].reshape(OC, P).T, dtype=np.float32)
        )

    xf = np.ascontiguousarray(x).reshape(ROWS, IN_F)
    in_maps = []
    for k in range(NCORES):
        b = (k * ROWS_PER_CORE) // N  # batch this core's rows belong to
        xT = np.empty((IN_F, ROWS_PER_CORE), dtype=np.float16)
        np.copyto(xT, xf[k * ROWS_PER_CORE : (k + 1) * ROWS_PER_CORE].T)
        in_maps.append({"xt": xT, "wt": wpack[b], "betat": btpack[b]})
    return in_maps


def kernel(x, z, weight, weight_alpha, bias_alpha, weight_beta, bias_beta,
           _trace=False):
    from concourse.bass_utils import run_bass_kernel_spmd

    x = np.asarray(x, dtype=np.float32)
    z = np.asarray(z, dtype=np.float32)
    weight = np.asarray(weight, dtype=np.float32)
    weight_alpha = np.asarray(weight_alpha, dtype=np.float32)
    bias_alpha = np.asarray(bias_alpha, dtype=np.float32)
    weight_beta = np.asarray(weight_beta, dtype=np.float32)
    bias_beta = np.asarray(bias_beta, dtype=np.float32)
    in_maps = host_prep(
        x, z, weight, weight_alpha, bias_alpha, weight_beta, bias_beta
    )
    nc = _get_nc()
    res = run_bass_kernel_spmd(
        nc, in_maps, core_ids=list(range(NCORES)), trace=_trace
    )
    out = np.empty((ROWS, OUT_F), dtype=np.float32)
    for k in range(NCORES):
        np.copyto(
            out[k * ROWS_PER_CORE : (k + 1) * ROWS_PER_CORE],
            res.results[k]["out"].T,
        )
    out = out.reshape(B, N, OUT_F)
    if _trace:
        kernel.last_results = res
    return out
